# revision 4
# baseline (speedup 1.0000x reference)
"""Trainium2 Bass kernel for nn_CausalGraphGenerator.

Reference semantics: the per-channel conv predictor is channel-separable, so
the influence matrix A[b] is diagonal. Hence A - A^T == 0 identically and

    adj[b, i, j] = relu(0 - h) = max(-h, 0)   for i != j
    adj[b, i, i] = 0

for ANY X / conv weights — the output depends only on the scalar threshold h.
(Verified numerically against the reference, including h < 0 and perturbed X.)

Device kernel (SPMD on 8 NeuronCores, batch-parallel: core b produces batch
b's [C, C] adjacency slice):
    out = max(negmask * h, 0)
with negmask = -(1 - I) and h packed into one [C, C+1] input (col 0 = h
replicated per partition — the per-partition scalar operand of a single
VectorE tensor_scalar instruction; cols 1..C = negmask). Since
negmask ∈ {-1, 0}, max(negmask * h, 0) == (1 - I) * relu(-h) exactly.

Raw Bass (no TileContext, no Block): the in-DMA issues from the ACT HWDGE
queue, the one tensor_scalar runs on DVE, the out-DMA from the SP HWDGE queue
(pre-armed on the compute semaphore), with sem waits attached directly to the
consuming instructions. This avoids Tile's kernel-tail drain (whose >2 sem
waits the neuronx-cc CoreV3 codegen used by the bass2jax/PJRT path rejects:
"Too many sync wait commands"), Tile's all-engine barrier epilogue, and the
Block-exit barrier. Bass's BIR preamble (register movs / const memsets /
all-engine barrier) is stripped after tracing — see _strip_preamble. Validated
in CoreSim (race detector) and on HW across repeated executions with varying h
(semaphores are reset per execution by the runtime). Measured ~8.3 us/core on
HW (stable +/-30 ns); ~2.4 us of that is the kernel body, the rest is fixed
walrus/BSP scaffolding (engine-start events — the PE engine's start event
arrives ~3 us late, gating the start barrier — base-register TENSOR_LOADs,
and inter-preamble barriers) that exists for any NEFF on this path.
"""

import os

import numpy as np

_B, _W, _C = 4, 2048, 64
_N_CORES = 8

# Number of SBUF partitions the [C, C] problem is folded onto. Each DMA
# needs one descriptor per partition, so fewer partitions cut both DMAs'
# descriptor-generation time and the completion-semaphore fan-in; the DVE
# tensor_scalar conversely processes C*C/P elements per partition. P=16
# was fastest on HW (descriptor gen dominates the DVE element cost).
_P = int(os.environ.get("BASS_P", "16"))
_F = (_C * _C) // _P  # free elems per partition
_DROP_POOL_QUEUE = os.environ.get("BASS_DROP_POOL_QUEUE", "1") == "1"

_CACHE = {}


def _build_nc():
    """Build (once) the single-core Bass program run SPMD on all 8 cores."""
    if "nc" in _CACHE:
        return _CACHE["nc"]

    import concourse.bass as bass
    import concourse.mybir as mybir

    nc = bass.Bass("TRN2", target_bir_lowering=False)

    packed_t = nc.dram_tensor(
        "packed", [_P, _F + 1], mybir.dt.float32, kind="ExternalInput"
    )
    out_t = nc.dram_tensor("out", [_P, _F], mybir.dt.float32, kind="ExternalOutput")

    with (
        nc.sbuf_tensor("pk", [_P, _F + 1], mybir.dt.float32) as pk,
        nc.sbuf_tensor("o", [_P, _F], mybir.dt.float32) as o,
        nc.semaphore("dma_sem") as dma_sem,
        nc.semaphore("comp_sem") as comp_sem,
    ):
        nc.scalar.dma_start(out=pk[:, :], in_=packed_t.ap()).then_inc(dma_sem, 16)
        nc.vector.tensor_scalar(
            out=o[:, :],
            in0=pk[:, 1 : _F + 1],
            scalar1=pk[:, 0:1],
            scalar2=0.0,
            op0=mybir.AluOpType.mult,
            op1=mybir.AluOpType.max,
        )._wait_ge(dma_sem, 16).then_inc(comp_sem, 1)
        # out-DMA on the otherwise-idle SP HWDGE queue: SP sits pre-armed on
        # comp_sem and fires the moment the tensor_scalar retires, and the
        # end-of-kernel queue drains then run on two engines in parallel
        # (measured ~160 ns faster than issuing both DMAs from ACT)
        nc.sync.dma_start(out=out_t.ap(), in_=o[:, :])._wait_ge(
            comp_sem, 1
        ).then_inc(dma_sem, 16)

    if _DROP_POOL_QUEUE:
        # The kernel only uses the two HWDGE queues (ACT + SP); the SWDGE
        # qPoolDynamic queue Bass declares by default just adds NEFF
        # queue-arming and teardown work. Guarded: fall back silently if a
        # concourse change makes the queue load-bearing.
        try:
            qs = [q for q in nc.m.queues if not q.name.startswith("qPoolDynamic")]
            if len(qs) < len(nc.m.queues):
                nc.m.queues = qs
        except Exception:
            pass

    _strip_preamble(nc)
    _CACHE["nc"] = nc
    return nc


def _strip_preamble(nc):
    """Drop Bass's BIR preamble (per-engine register movs, const-AP memsets,
    and the all-engine barrier) — none of it is used by this kernel's three
    instructions (the tensor_scalar's scalar2 lowers to an immediate, not a
    const AP). Measured saving: ~3.1 us/exec (11.5 -> 8.4 us). Guarded by an
    exact structural match so a concourse layout change falls back to the
    unstripped (still correct) program. Validated in CoreSim and on HW with
    varying h across repeated executions."""
    import concourse.mybir as mybir

    bb = nc.m.functions[0].blocks[0]
    insts = list(bb.instructions)
    strippable = (
        mybir.InstRegisterMove,
        mybir.InstMemset,
        mybir.InstDrain,
        mybir.InstEventSemaphore,
    )
    if (
        len(insts) >= 5
        and isinstance(insts[0], mybir.InstCall)
        and all(isinstance(i, strippable) for i in insts[1:-3])
        and isinstance(insts[-3], mybir.InstDMACopy)
        and isinstance(insts[-2], mybir.InstTensorScalarPtr)
        and isinstance(insts[-1], mybir.InstDMACopy)
    ):
        bb.instructions = [insts[0]] + insts[-3:]


def _in_map(h_value):
    hv = np.float32(np.asarray(h_value).reshape(()))
    packed = np.empty((_P, _F + 1), dtype=np.float32)
    packed[:, 0] = hv
    negmask = -(1.0 - np.eye(_C, dtype=np.float32))  # [C, C], row-major
    packed[:, 1:] = negmask.reshape(_P, _F)
    return {"packed": packed}


def _cached_exec():
    """One-time jit of the SPMD executable (same lowering as
    bass2jax.run_bass_via_pjrt's multi-core path); repeat kernel() calls
    then skip re-tracing and go straight to device execution."""
    if "exec" in _CACHE:
        return _CACHE["exec"]

    import jax
    import concourse.mybir as mybir
    from jax.sharding import Mesh, PartitionSpec
    from jax.experimental.shard_map import shard_map
    from concourse.bass2jax import (
        _bass_exec_p,
        install_neuronx_cc_hook,
        partition_id_tensor,
    )

    nc = _build_nc()
    install_neuronx_cc_hook()
    assert nc.dbg_addr is None
    partition_name = nc.partition_id_tensor.name if nc.partition_id_tensor else None

    in_names, out_names, out_avals, zero_outs = [], [], [], []
    for alloc in nc.m.functions[0].allocations:
        if not isinstance(alloc, mybir.MemoryLocationSet):
            continue
        name = alloc.memorylocations[0].name
        if alloc.kind == "ExternalInput":
            if name != partition_name:
                in_names.append(name)
        elif alloc.kind == "ExternalOutput":
            shape = tuple(alloc.tensor_shape)
            dtype = mybir.dt.np(alloc.dtype)
            out_names.append(name)
            out_avals.append(jax.core.ShapedArray(shape, dtype))
            zero_outs.append(np.zeros(shape, dtype))
    n_params = len(in_names)
    all_names = in_names + out_names + ([partition_name] if partition_name else [])

    def _body(*args):
        operands = list(args)
        if partition_name is not None:
            operands.append(partition_id_tensor())
        return tuple(
            _bass_exec_p.bind(
                *operands,
                out_avals=tuple(out_avals),
                in_names=tuple(all_names),
                out_names=tuple(out_names),
                lowering_input_output_aliases=(),
                sim_require_finite=True,
                sim_require_nnan=True,
                nc=nc,
            )
        )

    devices = jax.devices()[:_N_CORES]
    mesh = Mesh(np.asarray(devices), ("core",))
    n_outs = len(out_names)
    sharded = jax.jit(
        shard_map(
            _body,
            mesh=mesh,
            in_specs=(PartitionSpec("core"),) * (n_params + n_outs),
            out_specs=(PartitionSpec("core"),) * n_outs,
            check_rep=False,
        ),
        donate_argnums=tuple(range(n_params, n_params + n_outs)),
        keep_unused=True,
    )

    def run_spmd(in_maps):
        concat_in = [
            np.concatenate([m[name] for m in in_maps], axis=0) for name in in_names
        ]
        concat_zero = [
            np.zeros((_N_CORES * z.shape[0], *z.shape[1:]), z.dtype)
            for z in zero_outs
        ]
        out_arrs = sharded(*concat_in, *concat_zero)
        return [
            {
                name: np.asarray(out_arrs[i]).reshape(
                    _N_CORES, *out_avals[i].shape
                )[c]
                for i, name in enumerate(out_names)
            }
            for c in range(_N_CORES)
        ]

    _CACHE["exec"] = run_spmd
    return run_spmd


def run(h, trace=False):
    """Run the SPMD kernel on cores 0-7; returns (out [B,C,C], results)."""
    in_maps = [_in_map(h) for _ in range(_N_CORES)]
    if trace:
        from concourse.bass_utils import run_bass_kernel_spmd

        res = run_bass_kernel_spmd(
            _build_nc(), in_maps, list(range(_N_CORES)), trace=True
        )
        results = res.results
    else:
        res = None
        try:
            results = _cached_exec()(in_maps)
        except Exception:  # fall back to the stock (re-tracing) runner
            _CACHE.pop("exec", None)
            from concourse.bass_utils import run_bass_kernel_spmd

            results = run_bass_kernel_spmd(
                _build_nc(), in_maps, list(range(_N_CORES))
            ).results
    # Batch-parallel gather: batch b comes from core b. The device tensor is
    # [P, F] row-major == the [C, C] adjacency flattened row-major.
    out = np.stack(
        [results[b]["out"].reshape(_C, _C) for b in range(_B)], axis=0
    )
    return np.ascontiguousarray(out, dtype=np.float32), res


def kernel(X, w1, b1, w2, b2, h, **_unused):
    out, _ = run(h)
    return out



# revision 19
# speedup vs baseline: 1.0207x; 1.0207x over previous
"""Trainium2 Bass kernel for nn_CausalGraphGenerator.

Reference semantics: the per-channel conv predictor is channel-separable, so
the influence matrix A[b] is diagonal. Hence A - A^T == 0 identically and

    adj[b, i, j] = relu(0 - h) = max(-h, 0)   for i != j
    adj[b, i, i] = 0

for ANY X / conv weights — the output depends only on the scalar threshold h.
(Verified numerically against the reference, including h < 0 and perturbed X.)

Device kernel (SPMD on 8 NeuronCores, batch-parallel: core b produces batch
b's [C, C] adjacency slice):
    out = max(negmask * h, 0)
with negmask = -(1 - I) and h packed into one [C, C+1] input (col 0 = h
replicated per partition — the per-partition scalar operand of a single
VectorE tensor_scalar instruction; cols 1..C = negmask). Since
negmask ∈ {-1, 0}, max(negmask * h, 0) == (1 - I) * relu(-h) exactly.

Raw Bass (no TileContext, no Block): the in-DMA issues from the ACT HWDGE
queue, the one tensor_scalar runs on DVE, the out-DMA from the SP HWDGE queue
(pre-armed on the compute semaphore), with sem waits attached directly to the
consuming instructions. This avoids Tile's kernel-tail drain (whose >2 sem
waits the neuronx-cc CoreV3 codegen used by the bass2jax/PJRT path rejects:
"Too many sync wait commands"), Tile's all-engine barrier epilogue, and the
Block-exit barrier. Bass's BIR preamble (register movs / const memsets /
all-engine barrier) is stripped after tracing — see _strip_preamble. Validated
in CoreSim (race detector) and on HW across repeated executions with varying h
(semaphores are reset per execution by the runtime). Measured ~8.3 us/core on
HW (stable +/-30 ns); ~2.4 us of that is the kernel body, the rest is fixed
walrus/BSP scaffolding (engine-start events — the PE engine's start event
arrives ~3 us late, gating the start barrier — base-register TENSOR_LOADs,
and inter-preamble barriers) that exists for any NEFF on this path.
"""

import os

import numpy as np

_B, _W, _C = 4, 2048, 64
_N_CORES = 8

# Number of SBUF partitions the [C, C] problem is folded onto. Each DMA
# needs one descriptor per partition, so fewer partitions cut both DMAs'
# descriptor-generation time and the completion-semaphore fan-in; the DVE
# tensor_scalar conversely processes C*C/P elements per partition. P=16
# was fastest on HW (descriptor gen dominates the DVE element cost).
_P = int(os.environ.get("BASS_P", "64"))
_F = (_C * _C) // _P  # free elems per partition
_DROP_POOL_QUEUE = os.environ.get("BASS_DROP_POOL_QUEUE", "0") == "1"
_HWDGE_NQ = int(os.environ.get("BASS_HWDGE_NQ", "16"))
# "swdge": out-DMA descriptors pre-generated on the Pool engine (SWDGE
# kv_writeback prepare_only) with a cheap trigger after compute. Dead on this
# toolchain: walrus CoreV2 codegen rejects InstTriggerDma/InstIncSwdgeSem
# ("ISA wrong length"), so the default is the SP HWDGE dynamic out-DMA.
_OUT_MODE = os.environ.get("BASS_OUT_MODE", "hwdge")
# Split the in-DMA into two halves issued concurrently from the ACT and SP
# HWDGE queues (each +16 on dma_sem; the tensor_scalar waits >= 32). Measured
# neutral on HW (SP reaches its body ~950 ns after ACT, so its half lands
# last) — off by default.
_SPLIT_IN = os.environ.get("BASS_SPLIT_IN", "0") == "1"

_CACHE = {}


def _build_nc():
    """Build (once) the single-core Bass program run SPMD on all 8 cores."""
    if "nc" in _CACHE:
        return _CACHE["nc"]

    import concourse.bass as bass
    import concourse.mybir as mybir

    nc = bass.Bass("TRN2", target_bir_lowering=False)

    packed_t = nc.dram_tensor(
        "packed", [_P, _F + 1], mybir.dt.float32, kind="ExternalInput"
    )
    if _OUT_MODE == "swdge":
        assert _P == 64
        # [1, 64, 2, 32] row-major == the [64, 64] adjacency flattened.
        out_t = nc.dram_tensor(
            "out", [1, _C, 2, _C // 2], mybir.dt.float32, kind="ExternalOutput"
        )
    else:
        out_t = nc.dram_tensor(
            "out", [_P, _F], mybir.dt.float32, kind="ExternalOutput"
        )

    with (
        nc.sbuf_tensor("pk", [_P, _F + 1], mybir.dt.float32) as pk,
        nc.sbuf_tensor("o", [_P, _F], mybir.dt.float32) as o,
        nc.sbuf_tensor("ctxidx", [128, 1], mybir.dt.int32) as ctxidx,
        nc.semaphore("dma_sem") as dma_sem,
        nc.semaphore("out_sem") as out_sem,
        nc.semaphore("prep_sem") as prep_sem,
        nc.semaphore("comp_sem") as comp_sem,
    ):
        if _OUT_MODE == "swdge":
            # Pool engine: write the kv_writeback ctx index (0) and generate
            # the out-DMA descriptors into the SWDGE ring while the in-DMA is
            # still in flight. Only the trigger below stays on the critical
            # path after compute.
            nc.gpsimd.memset(ctxidx[:, :], 0)
            nc.gpsimd.kv_writeback(
                out_ap=out_t.ap(),
                in_ap=o.ap().rearrange("p (a b c) -> p a b c", a=2, b=1, c=_C // 2),
                ctx_idxs_ap=ctxidx[:, :],
                prepare_only=True,
                sem=out_sem,
            ).then_inc(prep_sem, 1)
        if _SPLIT_IN:
            half = _P // 2
            nc.scalar.dma_start(
                out=pk[:half, :], in_=packed_t.ap()[:half, :]
            ).then_inc(dma_sem, 16)
            nc.sync.dma_start(
                out=pk[half:, :], in_=packed_t.ap()[half:, :]
            ).then_inc(dma_sem, 16)
            in_done = 32
        else:
            nc.scalar.dma_start(out=pk[:, :], in_=packed_t.ap()).then_inc(
                dma_sem, 16
            )
            in_done = 16
        nc.vector.tensor_scalar(
            out=o[:, :],
            in0=pk[:, 1 : _F + 1],
            scalar1=pk[:, 0:1],
            scalar2=0.0,
            op0=mybir.AluOpType.mult,
            op1=mybir.AluOpType.max,
        )._wait_ge(dma_sem, in_done).then_inc(comp_sem, 1)
        if _OUT_MODE == "swdge":
            # Standalone wait: descriptors committed to the SWDGE ring. This
            # retires during the in-DMA landing window, so only the comp_sem
            # wait + trigger write remain after the tensor_scalar.
            nc.gpsimd.wait_ge(prep_sem, 1)
            nc.gpsimd.trigger_dma(count=1)._wait_ge(comp_sem, 1)
        else:
            # out-DMA on the otherwise-idle SP HWDGE queue: SP sits pre-armed
            # on comp_sem and fires the moment the tensor_scalar retires, and
            # the end-of-kernel queue drains then run on two engines in
            # parallel (measured ~160 ns faster than issuing both from ACT).
            # A detached wait_ge + wait-free DMA was tried and is NOT faster
            # (the ~650 ns DMA_DIRECT2D engine cost is DGE descriptor-gen,
            # not wait accounting) and once wedged the core (status 101).
            nc.sync.dma_start(out=out_t.ap(), in_=o[:, :])._wait_ge(
                comp_sem, 1
            ).then_inc(dma_sem, 16)

    if _HWDGE_NQ != 16:
        for q in nc.m.queues:
            if getattr(q, "is_HWDGE", False):
                q.num_queues = _HWDGE_NQ

    if _DROP_POOL_QUEUE:
        # The kernel only uses the two HWDGE queues (ACT + SP); the SWDGE
        # qPoolDynamic queue Bass declares by default just adds NEFF
        # queue-arming and teardown work. Guarded: fall back silently if a
        # concourse change makes the queue load-bearing.
        try:
            qs = [q for q in nc.m.queues if not q.name.startswith("qPoolDynamic")]
            if len(qs) < len(nc.m.queues):
                nc.m.queues = qs
        except Exception:
            pass

    _strip_preamble(nc)
    _CACHE["nc"] = nc
    return nc


def _strip_preamble(nc):
    """Drop Bass's BIR preamble (per-engine register movs, const-AP memsets,
    and the all-engine barrier) — none of it is used by this kernel's three
    instructions (the tensor_scalar's scalar2 lowers to an immediate, not a
    const AP). Measured saving: ~3.1 us/exec (11.5 -> 8.4 us). Guarded by an
    exact structural match so a concourse layout change falls back to the
    unstripped (still correct) program. Validated in CoreSim and on HW with
    varying h across repeated executions."""
    import concourse.mybir as mybir

    import concourse.bass_isa as bass_isa

    bb = nc.m.functions[0].blocks[0]
    insts = list(bb.instructions)
    strippable = (
        mybir.InstRegisterMove,
        mybir.InstMemset,
        mybir.InstDrain,
        mybir.InstEventSemaphore,
    )
    if _OUT_MODE == "swdge":
        tail_types = [
            mybir.InstMemset,  # ctxidx = 0 (ours — keep)
            mybir.InstKVWritebackAnt,
            mybir.InstDMACopy,
            mybir.InstTensorScalarPtr,
            mybir.InstEventSemaphore,  # wait_ge(prep_sem) — keep
            bass_isa.InstTriggerDma,
        ]
    else:
        tail_types = [
            mybir.InstDMACopy,
            mybir.InstTensorScalarPtr,
            mybir.InstDMACopy,
        ]
        if _SPLIT_IN:
            tail_types.insert(0, mybir.InstDMACopy)
    k = len(tail_types)
    if (
        len(insts) >= k + 2
        and isinstance(insts[0], mybir.InstCall)
        and all(isinstance(i, strippable) for i in insts[1:-k])
        and all(isinstance(i, t) for i, t in zip(insts[-k:], tail_types))
    ):
        bb.instructions = [insts[0]] + insts[-k:]


def _in_map(h_value):
    hv = np.float32(np.asarray(h_value).reshape(()))
    packed = np.empty((_P, _F + 1), dtype=np.float32)
    packed[:, 0] = hv
    negmask = -(1.0 - np.eye(_C, dtype=np.float32))  # [C, C], row-major
    packed[:, 1:] = negmask.reshape(_P, _F)
    return {"packed": packed}


def _cached_exec():
    """One-time jit of the SPMD executable (same lowering as
    bass2jax.run_bass_via_pjrt's multi-core path); repeat kernel() calls
    then skip re-tracing and go straight to device execution."""
    if "exec" in _CACHE:
        return _CACHE["exec"]

    import jax
    import concourse.mybir as mybir
    from jax.sharding import Mesh, PartitionSpec
    from jax.experimental.shard_map import shard_map
    from concourse.bass2jax import (
        _bass_exec_p,
        install_neuronx_cc_hook,
        partition_id_tensor,
    )

    nc = _build_nc()
    install_neuronx_cc_hook()
    assert nc.dbg_addr is None
    partition_name = nc.partition_id_tensor.name if nc.partition_id_tensor else None

    in_names, out_names, out_avals, zero_outs = [], [], [], []
    for alloc in nc.m.functions[0].allocations:
        if not isinstance(alloc, mybir.MemoryLocationSet):
            continue
        name = alloc.memorylocations[0].name
        if alloc.kind == "ExternalInput":
            if name != partition_name:
                in_names.append(name)
        elif alloc.kind == "ExternalOutput":
            shape = tuple(alloc.tensor_shape)
            dtype = mybir.dt.np(alloc.dtype)
            out_names.append(name)
            out_avals.append(jax.core.ShapedArray(shape, dtype))
            zero_outs.append(np.zeros(shape, dtype))
    n_params = len(in_names)
    all_names = in_names + out_names + ([partition_name] if partition_name else [])

    def _body(*args):
        operands = list(args)
        if partition_name is not None:
            operands.append(partition_id_tensor())
        return tuple(
            _bass_exec_p.bind(
                *operands,
                out_avals=tuple(out_avals),
                in_names=tuple(all_names),
                out_names=tuple(out_names),
                lowering_input_output_aliases=(),
                sim_require_finite=True,
                sim_require_nnan=True,
                nc=nc,
            )
        )

    devices = jax.devices()[:_N_CORES]
    mesh = Mesh(np.asarray(devices), ("core",))
    n_outs = len(out_names)
    sharded = jax.jit(
        shard_map(
            _body,
            mesh=mesh,
            in_specs=(PartitionSpec("core"),) * (n_params + n_outs),
            out_specs=(PartitionSpec("core"),) * n_outs,
            check_rep=False,
        ),
        donate_argnums=tuple(range(n_params, n_params + n_outs)),
        keep_unused=True,
    )

    def run_spmd(in_maps):
        concat_in = [
            np.concatenate([m[name] for m in in_maps], axis=0) for name in in_names
        ]
        concat_zero = [
            np.zeros((_N_CORES * z.shape[0], *z.shape[1:]), z.dtype)
            for z in zero_outs
        ]
        out_arrs = sharded(*concat_in, *concat_zero)
        return [
            {
                name: np.asarray(out_arrs[i]).reshape(
                    _N_CORES, *out_avals[i].shape
                )[c]
                for i, name in enumerate(out_names)
            }
            for c in range(_N_CORES)
        ]

    _CACHE["exec"] = run_spmd
    return run_spmd


def run(h, trace=False):
    """Run the SPMD kernel on cores 0-7; returns (out [B,C,C], results)."""
    in_maps = [_in_map(h) for _ in range(_N_CORES)]
    if trace:
        from concourse.bass_utils import run_bass_kernel_spmd

        res = run_bass_kernel_spmd(
            _build_nc(), in_maps, list(range(_N_CORES)), trace=True
        )
        results = res.results
    else:
        res = None
        try:
            results = _cached_exec()(in_maps)
        except Exception:  # fall back to the stock (re-tracing) runner
            _CACHE.pop("exec", None)
            from concourse.bass_utils import run_bass_kernel_spmd

            results = run_bass_kernel_spmd(
                _build_nc(), in_maps, list(range(_N_CORES))
            ).results
    # Batch-parallel gather: batch b comes from core b. The device tensor is
    # [P, F] row-major == the [C, C] adjacency flattened row-major.
    out = np.stack(
        [results[b]["out"].reshape(_C, _C) for b in range(_B)], axis=0
    )
    return np.ascontiguousarray(out, dtype=np.float32), res


def kernel(X, w1, b1, w2, b2, h, **_unused):
    out, _ = run(h)
    return out



# revision 21
# speedup vs baseline: 1.0210x; 1.0004x over previous
"""Trainium2 Bass kernel for nn_CausalGraphGenerator.

Reference semantics: the per-channel conv predictor is channel-separable, so
the influence matrix A[b] is diagonal. Hence A - A^T == 0 identically and

    adj[b, i, j] = relu(0 - h) = max(-h, 0)   for i != j
    adj[b, i, i] = 0

for ANY X / conv weights — the output depends only on the scalar threshold h.
(Verified numerically against the reference, including h < 0 and perturbed X.)

Device kernel (SPMD on 8 NeuronCores, batch-parallel: core b produces batch
b's [C, C] adjacency slice):
    out = max(negmask * h, 0)
with negmask = -(1 - I) and h packed into one [C, C+1] input (col 0 = h
replicated per partition — the per-partition scalar operand of a single
VectorE tensor_scalar instruction; cols 1..C = negmask). Since
negmask ∈ {-1, 0}, max(negmask * h, 0) == (1 - I) * relu(-h) exactly.

Raw Bass (no TileContext, no Block): the in-DMA issues from the ACT HWDGE
queue, the one tensor_scalar runs on DVE, the out-DMA from the SP HWDGE queue
(pre-armed on the compute semaphore), with sem waits attached directly to the
consuming instructions. This avoids Tile's kernel-tail drain (whose >2 sem
waits the neuronx-cc CoreV3 codegen used by the bass2jax/PJRT path rejects:
"Too many sync wait commands"), Tile's all-engine barrier epilogue, and the
Block-exit barrier. Bass's BIR preamble (register movs / const memsets /
all-engine barrier) is stripped after tracing — see _strip_preamble. Validated
in CoreSim (race detector) and on HW across repeated executions with varying h
(semaphores are reset per execution by the runtime). Measured ~8.3 us/core on
HW (stable +/-30 ns); ~2.4 us of that is the kernel body, the rest is fixed
walrus/BSP scaffolding (engine-start events — the PE engine's start event
arrives ~3 us late, gating the start barrier — base-register TENSOR_LOADs,
and inter-preamble barriers) that exists for any NEFF on this path.
"""

import os

import numpy as np

_B, _W, _C = 4, 2048, 64
_N_CORES = 8

# Number of SBUF partitions the [C, C] problem is folded onto. Each DMA
# needs one descriptor per partition, so fewer partitions cut both DMAs'
# descriptor-generation time and the completion-semaphore fan-in; the DVE
# tensor_scalar conversely processes C*C/P elements per partition. P=16
# was fastest on HW (descriptor gen dominates the DVE element cost).
_P = int(os.environ.get("BASS_P", "64"))
_F = (_C * _C) // _P  # free elems per partition
_DROP_POOL_QUEUE = os.environ.get("BASS_DROP_POOL_QUEUE", "0") == "1"
_HWDGE_NQ = int(os.environ.get("BASS_HWDGE_NQ", "16"))
# "swdge": out-DMA descriptors pre-generated on the Pool engine (SWDGE
# kv_writeback prepare_only) with a cheap trigger after compute. Dead on this
# toolchain: walrus CoreV2 codegen rejects InstTriggerDma/InstIncSwdgeSem
# ("ISA wrong length"), so the default is the SP HWDGE dynamic out-DMA.
_OUT_MODE = os.environ.get("BASS_OUT_MODE", "hwdge")
# Split the in-DMA into two halves issued concurrently from the ACT and SP
# HWDGE queues (each +16 on dma_sem; the tensor_scalar waits >= 32). Measured
# neutral on HW (SP reaches its body ~950 ns after ACT, so its half lands
# last) — off by default.
_SPLIT_IN = os.environ.get("BASS_SPLIT_IN", "0") == "1"

_CACHE = {}


def _build_nc():
    """Build (once) the single-core Bass program run SPMD on all 8 cores."""
    if "nc" in _CACHE:
        return _CACHE["nc"]

    import concourse.bass as bass
    import concourse.mybir as mybir

    nc = bass.Bass("TRN2", target_bir_lowering=False)

    packed_t = nc.dram_tensor(
        "packed", [_P, _F + 1], mybir.dt.float32, kind="ExternalInput"
    )
    if _OUT_MODE == "swdge":
        assert _P == 64
        # [1, 64, 2, 32] row-major == the [64, 64] adjacency flattened.
        out_t = nc.dram_tensor(
            "out", [1, _C, 2, _C // 2], mybir.dt.float32, kind="ExternalOutput"
        )
    else:
        out_t = nc.dram_tensor(
            "out", [_P, _F], mybir.dt.float32, kind="ExternalOutput"
        )

    with (
        nc.sbuf_tensor("pk", [_P, _F + 1], mybir.dt.float32) as pk,
        nc.sbuf_tensor("o", [_P, _F], mybir.dt.float32) as o,
        nc.sbuf_tensor("ctxidx", [128, 1], mybir.dt.int32) as ctxidx,
        nc.semaphore("dma_sem") as dma_sem,
        nc.semaphore("out_sem") as out_sem,
        nc.semaphore("prep_sem") as prep_sem,
        nc.semaphore("comp_sem") as comp_sem,
    ):
        if _OUT_MODE == "swdge":
            # Pool engine: write the kv_writeback ctx index (0) and generate
            # the out-DMA descriptors into the SWDGE ring while the in-DMA is
            # still in flight. Only the trigger below stays on the critical
            # path after compute.
            nc.gpsimd.memset(ctxidx[:, :], 0)
            nc.gpsimd.kv_writeback(
                out_ap=out_t.ap(),
                in_ap=o.ap().rearrange("p (a b c) -> p a b c", a=2, b=1, c=_C // 2),
                ctx_idxs_ap=ctxidx[:, :],
                prepare_only=True,
                sem=out_sem,
            ).then_inc(prep_sem, 1)
        if _SPLIT_IN:
            half = _P // 2
            nc.scalar.dma_start(
                out=pk[:half, :], in_=packed_t.ap()[:half, :]
            ).then_inc(dma_sem, 16)
            nc.sync.dma_start(
                out=pk[half:, :], in_=packed_t.ap()[half:, :]
            ).then_inc(dma_sem, 16)
            in_done = 32
        else:
            nc.scalar.dma_start(out=pk[:, :], in_=packed_t.ap()).then_inc(
                dma_sem, 16
            )
            in_done = 16
        # negmask occupies cols 0..F-1 (offset-0, aligned AP); h sits in the
        # last column as the per-partition scalar operand.
        nc.vector.tensor_scalar(
            out=o[:, :],
            in0=pk[:, 0:_F],
            scalar1=pk[:, _F : _F + 1],
            scalar2=0.0,
            op0=mybir.AluOpType.mult,
            op1=mybir.AluOpType.max,
        )._wait_ge(dma_sem, in_done).then_inc(comp_sem, 1)
        if _OUT_MODE == "swdge":
            # Standalone wait: descriptors committed to the SWDGE ring. This
            # retires during the in-DMA landing window, so only the comp_sem
            # wait + trigger write remain after the tensor_scalar.
            nc.gpsimd.wait_ge(prep_sem, 1)
            nc.gpsimd.trigger_dma(count=1)._wait_ge(comp_sem, 1)
        else:
            # out-DMA on the otherwise-idle SP HWDGE queue: SP sits pre-armed
            # on comp_sem and fires the moment the tensor_scalar retires, and
            # the end-of-kernel queue drains then run on two engines in
            # parallel (measured ~160 ns faster than issuing both from ACT).
            # A detached wait_ge + wait-free DMA was tried and is NOT faster
            # (the ~650 ns DMA_DIRECT2D engine cost is DGE descriptor-gen,
            # not wait accounting) and once wedged the core (status 101).
            nc.sync.dma_start(out=out_t.ap(), in_=o[:, :])._wait_ge(
                comp_sem, 1
            ).then_inc(dma_sem, 16)

    if _HWDGE_NQ != 16:
        for q in nc.m.queues:
            if getattr(q, "is_HWDGE", False):
                q.num_queues = _HWDGE_NQ

    if _DROP_POOL_QUEUE:
        # The kernel only uses the two HWDGE queues (ACT + SP); the SWDGE
        # qPoolDynamic queue Bass declares by default just adds NEFF
        # queue-arming and teardown work. Guarded: fall back silently if a
        # concourse change makes the queue load-bearing.
        try:
            qs = [q for q in nc.m.queues if not q.name.startswith("qPoolDynamic")]
            if len(qs) < len(nc.m.queues):
                nc.m.queues = qs
        except Exception:
            pass

    _strip_preamble(nc)
    _CACHE["nc"] = nc
    return nc


def _strip_preamble(nc):
    """Drop Bass's BIR preamble (per-engine register movs, const-AP memsets,
    and the all-engine barrier) — none of it is used by this kernel's three
    instructions (the tensor_scalar's scalar2 lowers to an immediate, not a
    const AP). Measured saving: ~3.1 us/exec (11.5 -> 8.4 us). Guarded by an
    exact structural match so a concourse layout change falls back to the
    unstripped (still correct) program. Validated in CoreSim and on HW with
    varying h across repeated executions."""
    import concourse.mybir as mybir

    import concourse.bass_isa as bass_isa

    bb = nc.m.functions[0].blocks[0]
    insts = list(bb.instructions)
    strippable = (
        mybir.InstRegisterMove,
        mybir.InstMemset,
        mybir.InstDrain,
        mybir.InstEventSemaphore,
    )
    if _OUT_MODE == "swdge":
        tail_types = [
            mybir.InstMemset,  # ctxidx = 0 (ours — keep)
            mybir.InstKVWritebackAnt,
            mybir.InstDMACopy,
            mybir.InstTensorScalarPtr,
            mybir.InstEventSemaphore,  # wait_ge(prep_sem) — keep
            bass_isa.InstTriggerDma,
        ]
    else:
        tail_types = [
            mybir.InstDMACopy,
            mybir.InstTensorScalarPtr,
            mybir.InstDMACopy,
        ]
        if _SPLIT_IN:
            tail_types.insert(0, mybir.InstDMACopy)
    k = len(tail_types)
    if (
        len(insts) >= k + 2
        and isinstance(insts[0], mybir.InstCall)
        and all(isinstance(i, strippable) for i in insts[1:-k])
        and all(isinstance(i, t) for i, t in zip(insts[-k:], tail_types))
    ):
        bb.instructions = [insts[0]] + insts[-k:]


def _in_map(h_value):
    hv = np.float32(np.asarray(h_value).reshape(()))
    packed = np.empty((_P, _F + 1), dtype=np.float32)
    negmask = -(1.0 - np.eye(_C, dtype=np.float32))  # [C, C], row-major
    packed[:, :_F] = negmask.reshape(_P, _F)
    packed[:, _F] = hv
    return {"packed": packed}


def _cached_exec():
    """One-time jit of the SPMD executable (same lowering as
    bass2jax.run_bass_via_pjrt's multi-core path); repeat kernel() calls
    then skip re-tracing and go straight to device execution."""
    if "exec" in _CACHE:
        return _CACHE["exec"]

    import jax
    import concourse.mybir as mybir
    from jax.sharding import Mesh, PartitionSpec
    from jax.experimental.shard_map import shard_map
    from concourse.bass2jax import (
        _bass_exec_p,
        install_neuronx_cc_hook,
        partition_id_tensor,
    )

    nc = _build_nc()
    install_neuronx_cc_hook()
    assert nc.dbg_addr is None
    partition_name = nc.partition_id_tensor.name if nc.partition_id_tensor else None

    in_names, out_names, out_avals, zero_outs = [], [], [], []
    for alloc in nc.m.functions[0].allocations:
        if not isinstance(alloc, mybir.MemoryLocationSet):
            continue
        name = alloc.memorylocations[0].name
        if alloc.kind == "ExternalInput":
            if name != partition_name:
                in_names.append(name)
        elif alloc.kind == "ExternalOutput":
            shape = tuple(alloc.tensor_shape)
            dtype = mybir.dt.np(alloc.dtype)
            out_names.append(name)
            out_avals.append(jax.core.ShapedArray(shape, dtype))
            zero_outs.append(np.zeros(shape, dtype))
    n_params = len(in_names)
    all_names = in_names + out_names + ([partition_name] if partition_name else [])

    def _body(*args):
        operands = list(args)
        if partition_name is not None:
            operands.append(partition_id_tensor())
        return tuple(
            _bass_exec_p.bind(
                *operands,
                out_avals=tuple(out_avals),
                in_names=tuple(all_names),
                out_names=tuple(out_names),
                lowering_input_output_aliases=(),
                sim_require_finite=True,
                sim_require_nnan=True,
                nc=nc,
            )
        )

    devices = jax.devices()[:_N_CORES]
    mesh = Mesh(np.asarray(devices), ("core",))
    n_outs = len(out_names)
    sharded = jax.jit(
        shard_map(
            _body,
            mesh=mesh,
            in_specs=(PartitionSpec("core"),) * (n_params + n_outs),
            out_specs=(PartitionSpec("core"),) * n_outs,
            check_rep=False,
        ),
        donate_argnums=tuple(range(n_params, n_params + n_outs)),
        keep_unused=True,
    )

    def run_spmd(in_maps):
        concat_in = [
            np.concatenate([m[name] for m in in_maps], axis=0) for name in in_names
        ]
        concat_zero = [
            np.zeros((_N_CORES * z.shape[0], *z.shape[1:]), z.dtype)
            for z in zero_outs
        ]
        out_arrs = sharded(*concat_in, *concat_zero)
        return [
            {
                name: np.asarray(out_arrs[i]).reshape(
                    _N_CORES, *out_avals[i].shape
                )[c]
                for i, name in enumerate(out_names)
            }
            for c in range(_N_CORES)
        ]

    _CACHE["exec"] = run_spmd
    return run_spmd


def run(h, trace=False):
    """Run the SPMD kernel on cores 0-7; returns (out [B,C,C], results)."""
    in_maps = [_in_map(h) for _ in range(_N_CORES)]
    if trace:
        from concourse.bass_utils import run_bass_kernel_spmd

        res = run_bass_kernel_spmd(
            _build_nc(), in_maps, list(range(_N_CORES)), trace=True
        )
        results = res.results
    else:
        res = None
        try:
            results = _cached_exec()(in_maps)
        except Exception:  # fall back to the stock (re-tracing) runner
            _CACHE.pop("exec", None)
            from concourse.bass_utils import run_bass_kernel_spmd

            results = run_bass_kernel_spmd(
                _build_nc(), in_maps, list(range(_N_CORES))
            ).results
    # Batch-parallel gather: batch b comes from core b. The device tensor is
    # [P, F] row-major == the [C, C] adjacency flattened row-major.
    out = np.stack(
        [results[b]["out"].reshape(_C, _C) for b in range(_B)], axis=0
    )
    return np.ascontiguousarray(out, dtype=np.float32), res


def kernel(X, w1, b1, w2, b2, h, **_unused):
    out, _ = run(h)
    return out

